# revision 27
# baseline (speedup 1.0000x reference)
"""AdaptiveModulatedConv3d — 8-core TRN2 Bass kernel.

Problem (hardcoded): BS=8, C_IN=C_OUT=64, K=3, STYLE_DIM=512, BANK=4,
D=H=W=32, pad=1, stride=1, f32 in/out.

Sharding: pure data-parallel over batch — each of the 8 NeuronCores gets one
sample, builds its per-sample demodulated conv weights on-device, and runs
its own 3D conv. No collectives.

Per-core conv: 3x3x3 conv as 27 shifted matmuls (contraction over C_IN=64)
accumulating into PSUM, with the PE array quadrant-packed 2x2 (row groups =
two d-plane halves of x, col groups = two output tiles per group), so four
64x64 matmuls stream concurrently.

Layout/schedule notes (156us -> ~139us vs the previous kernel):
 - split-plane x: partitions 0-63 hold d-planes 0..16, partitions 64-127
   hold planes 15..31 (no shifted second copy; full-width 128-partition
   casts; per-plane staging chunks pipelined with the conv).
 - padded planes are 34x33 (one shared pad column per row: the left pad
   of row r doubles as the right guard of row r-1), cutting every
   matmul's free dim by 3% vs 34x34.
 - params consolidated into one DMA; bank mixed on the lower half only
   in 4 tap-chunks (WT upper half via SBUF->SBUF copies on the ACT DMA
   queue — each DMA queue has ONE counting completion semaphore, so a
   consumer of the Nth DMA waits for all earlier DMAs on that queue;
   the ACT queue only carries small early transfers).
 - demod from a host-side Gram matrix P[ci,n,m,co]: 16 tiny matmuls
   produce dmT[co,1] directly in partition layout, off the critical path.
 - the first 4 conv groups run their taps in 4 passes tracking the mix
   chunks, so the conv starts before WT is fully built; their drains are
   interleaved per group.
 - drains strip the pad columns (strided PSUM read -> contiguous SBUF),
   so output DMAs are fully contiguous.
"""

import numpy as np

import concourse.bass as bass
import concourse.tile as tile
from concourse import bacc, mybir
from concourse import bass_utils

F32 = mybir.dt.float32
BF16 = mybir.dt.bfloat16

BS = 8
CI = 64
CO = 64
SD = 512
BANK = 4
D = H = W = 32
EPS = 1e-8
NCORES = 8

# one padded plane: 34 rows x 33 cols -- a single pad column per row (the
# left pad of row r doubles as the right guard of row r-1)
PLANE = (H + 2) * (W + 1)          # 1122
LP = 17                            # d-planes stored per partition half
XLEN = 2 + LP * PLANE + 2          # front guard + planes + back guard
ROWSPLIT = [(0, 11), (11, 11), (22, 10)]   # h-row tiles per d-plane
KCH = [(0, 5), (5, 11), (11, 18), (18, 27)]  # mix tap-chunks (koff)
# par packing offsets (see _shard_inputs)
PAR_WK, PAR_FW, PAR_MWT, PAR_MB, PAR_FB, PAR_LEN = 0, 4, 20, 276, 277, 281

_CACHE = {}


def _conv_offsets(d):
    """Valid (kd, kh, kw) taps for output d-plane d, koff-ascending."""
    offs = []
    for kd in range(3):
        if 0 <= d + kd - 1 <= D - 1:
            for kh in range(3):
                for kw in range(3):
                    offs.append((kd, kh, kw))
    return offs


def _build():
    nc = bacc.Bacc("TRN2", target_bir_lowering=False, debug=False)
    x = nc.dram_tensor("x", [CI, D, H, W], F32, kind="ExternalInput").ap()
    par = nc.dram_tensor("par", [128, PAR_LEN], F32,
                         kind="ExternalInput").ap()
    bankt = nc.dram_tensor("bankt", [CI, BANK, 27 * CO], BF16,
                           kind="ExternalInput").ap()
    gram = nc.dram_tensor("gram", [CI, BANK, BANK, CO], BF16,
                          kind="ExternalInput").ap()
    out = nc.dram_tensor("out", [CO, D, H, W], F32, kind="ExternalOutput").ap()

    with tile.TileContext(nc) as tc:
        with tc.tile_pool(name="singles", bufs=1) as sg, \
             tc.tile_pool(name="stg", bufs=6) as stg_pool, \
             tc.tile_pool(name="osb", bufs=1) as osb_pool:

            xbf = sg.tile([128, XLEN], BF16)
            dmT = sg.tile([CO, 1], F32)
            pl = xbf[:, 2:2 + LP * PLANE].rearrange(
                "p (d h w) -> p d h w", h=H + 2, w=W + 1)
            stg_tiles = {}

            def x_chunk_dma(c, eng_lo, eng_hi):
                stg = stg_pool.tile([128, H, W], F32)
                eng_lo.dma_start(out=stg[0:64], in_=x[:, c])
                eng_hi.dma_start(out=stg[64:128], in_=x[:, 15 + c])
                stg_tiles[c] = stg

            def x_chunk_cast(c, eng):
                dst = pl[:, c, 1:H + 1, 1:W + 1]
                if eng is nc.scalar:
                    nc.scalar.copy(dst, stg_tiles.pop(c))
                else:
                    eng.tensor_copy(dst, stg_tiles.pop(c))

            # ---- t~0: x chunk 0 triggers ride ACT (idle until the exp);
            # ACT preloads the Exp table ----
            x_chunk_dma(0, nc.scalar, nc.scalar)
            warm = sg.tile([1, 1], F32)
            nc.vector.memset(warm, 0.0)
            nc.scalar.activation(warm, warm,
                                 mybir.ActivationFunctionType.Exp)
            ones1 = sg.tile([1, 128], F32)
            nc.vector.memset(ones1, 1.0)
            eps64 = sg.tile([CI, 1], F32)
            nc.vector.memset(eps64, EPS)
            nc.gpsimd.memset(xbf[:, 0:2], 0.0)
            nc.gpsimd.memset(xbf[:, 2 + LP * PLANE:XLEN], 0.0)
            nc.gpsimd.memset(pl[:, :, 0, :], 0.0)
            nc.gpsimd.memset(pl[:, :, H + 1, :], 0.0)
            nc.vector.memset(pl[:, :, :, 0], 0.0)

            # ---- sync-queue DMAs, criticality-ordered ----
            par_sb = sg.tile([128, PAR_LEN], F32)
            nc.sync.dma_start(out=par_sb, in_=par)
            bank_sb = sg.tile([CI, BANK, 27 * CO], BF16)

            def bank_chunk_dma(ci_):
                k0, k1 = KCH[ci_][0] * CO, KCH[ci_][1] * CO
                nc.sync.dma_start(out=bank_sb[:, :, k0:k1],
                                  in_=bankt[:, :, k0:k1])

            bank_chunk_dma(0)
            P_sb = sg.tile([CI, BANK, BANK, CO], BF16)
            nc.sync.dma_start(out=P_sb, in_=gram)
            stg1 = stg_pool.tile([128, H, W], F32)
            nc.sync.dma_start(out=stg1[0:64], in_=x[:, 1])
            bank_chunk_dma(1)
            nc.sync.dma_start(out=stg1[64:128], in_=x[:, 16])
            stg_tiles[1] = stg1
            stg2 = stg_pool.tile([128, H, W], F32)
            nc.sync.dma_start(out=stg2[0:64], in_=x[:, 2])
            bank_chunk_dma(2)
            nc.sync.dma_start(out=stg2[64:128], in_=x[:, 17])
            stg_tiles[2] = stg2
            bank_chunk_dma(3)
            x_chunk_dma(3, nc.sync, nc.sync)
            x_chunk_dma(4, nc.sync, nc.sync)

            wk_sb = par_sb[:, PAR_WK:PAR_WK + 4]
            fw_sb = par_sb[:, PAR_FW:PAR_FW + 16].rearrange(
                "p (c b) -> p c b", b=BANK)
            mwt_sb = par_sb[:, PAR_MWT:PAR_MWT + 256].rearrange(
                "p (c i) -> p c i", i=CI)
            mb_sb = par_sb[0:CI, PAR_MB:PAR_MB + 1]
            fb_sb = par_sb[0:1, PAR_FB:PAR_FB + 4]

            with tc.tile_pool(name="wpsum", bufs=2, space="PSUM") as wpsum:
                # ---- logits = w @ filter_w.T; exp without softmax
                # normalization (uniform scale cancels through demod) ----
                ps_l = wpsum.tile([1, BANK], F32, tag="wps")
                for c in range(4):
                    nc.tensor.matmul(ps_l, lhsT=wk_sb[:, c:c + 1],
                                     rhs=fw_sb[:, c, :],
                                     start=(c == 0), stop=(c == 3))
                # ---- mod = w @ mod_w.T + mod_b (lower half only) ----
                ps_m = wpsum.tile([CI, 1], F32, tag="wps")
                for c in range(4):
                    nc.tensor.matmul(ps_m, lhsT=mwt_sb[:, c, :],
                                     rhs=wk_sb[:, c:c + 1],
                                     start=(c == 0), stop=(c == 3))

                logits = sg.tile([1, BANK], F32)
                nc.vector.tensor_add(logits, ps_l, fb_sb)
                fwt = sg.tile([1, BANK], F32)
                nc.scalar.activation(fwt, logits,
                                     mybir.ActivationFunctionType.Exp)
                mod_sb = sg.tile([CI, 1], F32)
                nc.vector.tensor_add(mod_sb, ps_m, mb_sb)

                # fwt broadcast across partitions
                ps_fb = wpsum.tile([64, BANK], F32, tag="wps")
                nc.tensor.matmul(ps_fb, lhsT=ones1[:, 0:64], rhs=fwt,
                                 start=True, stop=True)

                # mix coefficients coef[ci,n] = fwt_n * mod[ci], and demod
                # coefficients coefN[ci,n] = fwt_n * mod[ci]^2
                coef = sg.tile([CI, BANK], F32)
                nc.vector.tensor_scalar_mul(coef, ps_fb, mod_sb[:, 0:1])
                mod2 = sg.tile([CI, 1], F32)
                nc.vector.tensor_mul(mod2, mod_sb, mod_sb)
                coefN = sg.tile([CI, BANK], BF16)
                nc.vector.tensor_scalar_mul(coefN, ps_fb, mod2[:, 0:1])
                # coefNM[ci, 4m+n] = fwt_m * coefN[ci, n]
                coefNM = sg.tile([CI, BANK * BANK], BF16)
                for m in range(4):
                    nc.vector.tensor_scalar_mul(
                        coefNM[:, 4 * m:4 * m + 4], coefN,
                        ps_fb[:, m:m + 1])

                # ---- bank mix -> WT[0:64], 3 tap chunks on DVE; upper
                # half follows via SBUF->SBUF copies on the GpSimd queue ----
                WT = sg.tile([128, 27, CO], BF16)
                mixtmp = [sg.tile([CI, 9 * CO], F32, name=f"mxt{j}")
                          for j in range(4)]

                def mix_chunk(ci_):
                    c0, c1 = KCH[ci_]
                    f0, f1 = c0 * CO, c1 * CO
                    tmp = mixtmp[ci_][:, 0:f1 - f0]
                    nc.vector.tensor_scalar_mul(tmp, bank_sb[:, 0, f0:f1],
                                                coef[:, 0:1])
                    for n in range(1, 3):
                        nc.vector.scalar_tensor_tensor(
                            out=tmp, in0=bank_sb[:, n, f0:f1],
                            scalar=coef[:, n:n + 1], in1=tmp,
                            op0=mybir.AluOpType.mult,
                            op1=mybir.AluOpType.add)
                    nc.vector.scalar_tensor_tensor(
                        out=WT[0:64, c0:c1].rearrange("p k c -> p (k c)"),
                        in0=bank_sb[:, 3, f0:f1],
                        scalar=coef[:, 3:4], in1=tmp,
                        op0=mybir.AluOpType.mult, op1=mybir.AluOpType.add)
                    # the WT upper-half copy rides the ACT DMA queue: the
                    # gpsimd queue's shared DMA-completion semaphore would
                    # make its waiters (all rg64 weight loads) wait for the
                    # x-chunk DMAs scheduled ahead of it there
                    nc.scalar.dma_start(out=WT[64:128, c0:c1],
                                        in_=WT[0:64, c0:c1])

                x_chunk_cast(0, nc.scalar)
                mix_chunk(0)
                x_chunk_cast(1, nc.vector)
                mix_chunk(1)
                mix_chunk(2)
                mix_chunk(3)
                x_chunk_cast(2, nc.scalar)
                x_chunk_cast(3, nc.vector)
                x_chunk_cast(4, nc.vector)

                # demod, directly in [co, 1] partition layout:
                # dmT[co] = rsqrt(sum_{n,m,ci} coefNM[ci,nm] P[ci,n,m,co])
                psDT = wpsum.tile([CO, 1], F32, tag="wps")
                for m in range(4):
                    for n in range(4):
                        nc.tensor.matmul(
                            psDT, lhsT=P_sb[:, n, m, :],
                            rhs=coefNM[:, 4 * m + n:4 * m + n + 1],
                            start=(m == 0 and n == 0),
                            stop=(m == 3 and n == 3))
                sstdT = sg.tile([CO, 1], F32)
                nc.scalar.activation(sstdT, psDT,
                                     mybir.ActivationFunctionType.Sqrt,
                                     bias=eps64[:, 0:1])
                nc.vector.reciprocal(dmT, sstdT)


            # remaining staging: c5-c8 triggers on sync (fires ~14-18us),
            # c9+ on the GpSimd queue; all remaining casts on DVE
            for c in range(5, 9):
                x_chunk_dma(c, nc.sync, nc.sync)
            for c in range(9, LP):
                x_chunk_dma(c, nc.gpsimd, nc.gpsimd)
            for c in range(5, LP):
                x_chunk_cast(c, nc.vector)

            # ---- conv ----

            ltiles = [(0, d, r0, nr)
                      for d in list(range(1, 16)) + [0]
                      for (r0, nr) in ROWSPLIT]
            utiles = [(1, d, r0, nr)
                      for d in range(16, 32)
                      for (r0, nr) in ROWSPLIT]
            groups = [(ltiles[2 * i], utiles[2 * i],
                       ltiles[2 * i + 1], utiles[2 * i + 1])
                      for i in range(24)]
            # quadrant j: (rg, psum-tile idx, psum partition base); all four
            # PE quadrants (rg, pb) distinct for concurrent streaming
            quads = [(0, 0, 0), (64, 1, 0), (0, 0, 64), (64, 1, 64)]

            def emit_taps(group, pss, offs_l, state, klo, khi):
                nwaves = max(len(o) for o in offs_l)
                for i in range(nwaves):
                    for j, (up, d, r0, nr) in enumerate(group):
                        offs = offs_l[j]
                        if i >= len(offs):
                            continue
                        kd, kh, kw = offs[i]
                        koff = kd * 9 + kh * 3 + kw
                        if not (klo <= koff < khi):
                            continue
                        rg, pi, pb = quads[j]
                        slot = d + kd - 1 - (15 if up else 0)
                        off = 2 + slot * PLANE + (r0 + kh) * 33 + kw
                        n = nr * 33
                        nc.tensor.matmul(
                            pss[pi][pb:pb + 64, 0:n],
                            lhsT=WT[rg:rg + 64, koff, :],
                            rhs=xbf[rg:rg + 64, off:off + n],
                            start=(state[j] == 0),
                            stop=(state[j] == len(offs) - 1))
                        state[j] += 1

            def emit_drains(gi, group, pss, split=False):
                osbG = osb_pool.tile([128, 2, 352], F32,
                                     name=f"osb{gi % 4}")
                for j, (up, d, r0, nr) in enumerate(group):
                    rg, pi, pb = quads[j]
                    n = nr * 33
                    slotj = j % 2
                    dst = osbG[pb:pb + 64, slotj, 0:nr * W].rearrange(
                        "p (r w) -> p r w", w=W)
                    src = pss[pi][pb:pb + 64, 0:n].rearrange(
                        "p (r w) -> p r w", w=33)[:, :, 0:W]
                    if gi >= 22 and j % 2:
                        nc.vector.tensor_scalar_mul(dst, src, dmT[:, 0:1])
                    else:
                        nc.scalar.mul(dst, src, dmT[:, 0:1])
                    if gi >= 22:
                        eng = (nc.sync, nc.gpsimd, nc.scalar,
                               nc.gpsimd)[j]
                    else:
                        eng = nc.gpsimd if j % 2 else nc.sync
                    eng.dma_start(
                        out=out[:, d, r0:r0 + nr, :],
                        in_=osbG[pb:pb + 64, slotj, 0:nr * W].rearrange(
                            "p (r w) -> p r w", w=W))

            with tc.tile_pool(name="cpsum", bufs=8, space="PSUM") as cp:
                # early block: groups 0-3 in 4 tap passes tracking the
                # mix chunks (all 8 banks; dmT needs no psum here)
                early = groups[0:4]
                epss = [[cp.tile([128, 512], F32, tag="cps",
                                 name=f"cps{gi}_{j}") for j in range(2)]
                        for gi in range(4)]
                eoffs = [[_conv_offsets(t[1]) for t in g] for g in early]
                estate = [[0] * 4 for _ in range(4)]
                for pi_, (klo, khi) in enumerate(KCH):
                    last_pass = pi_ == len(KCH) - 1
                    for gi in range(4):
                        emit_taps(early[gi], epss[gi], eoffs[gi],
                                  estate[gi], klo, khi)
                        if last_pass:
                            emit_drains(gi, early[gi], epss[gi],
                                        split=True)

                for gi in range(4, 24):
                    group = groups[gi]
                    pss = [cp.tile([128, 512], F32, tag="cps",
                                   name=f"cps{gi % 4}_{j}")
                           for j in range(2)]
                    offs_l = [_conv_offsets(t[1]) for t in group]
                    state = [0] * 4
                    emit_taps(group, pss, offs_l, state, 0, 27)
                    emit_drains(gi, group, pss)

    nc.compile()
    return nc


def _shard_inputs(x, w, filter_w, filter_b, mod_w, mod_b, bank):
    """Host-side input marshalling: per-core shards + replicated params in
    the layouts the kernel expects."""
    import ml_dtypes
    fw_h = filter_w.T.reshape(4, 128, BANK).transpose(1, 0, 2)  # [128,4,4]
    mwt_h = mod_w.T.reshape(4, 128, CI).transpose(1, 0, 2)      # [128,4,64]
    bank_h = np.ascontiguousarray(
        bank.reshape(BANK, CO, CI, 27).transpose(2, 0, 3, 1)
        .reshape(CI, BANK, 27 * CO)).astype(ml_dtypes.bfloat16)
    # Gram matrix for the demodulation sum:
    # P[ci, n, m, co] = sum_k bank[n, co, ci, k] * bank[m, co, ci, k]
    bk = np.asarray(bank, np.float32).reshape(BANK, CO, CI, 27)
    gram_h = np.einsum('nuik,muik->inmu', bk, bk)
    gram_h = np.ascontiguousarray(gram_h).astype(ml_dtypes.bfloat16)
    par_base = np.zeros((128, PAR_LEN), np.float32)
    par_base[:, PAR_FW:PAR_FW + 16] = np.asarray(fw_h).reshape(128, 16)
    par_base[:, PAR_MWT:PAR_MWT + 256] = np.asarray(mwt_h).reshape(128, 256)
    par_base[0:CI, PAR_MB] = mod_b
    par_base[0, PAR_FB:PAR_FB + 4] = filter_b
    in_maps = []
    for i in range(NCORES):
        par_h = par_base.copy()
        par_h[:, PAR_WK:PAR_WK + 4] = w[i].reshape(4, 128).T
        in_maps.append({
            "x": np.ascontiguousarray(x[i], np.float32),
            "par": par_h, "bankt": bank_h, "gram": gram_h,
        })
    return in_maps


def _run(inputs, trace=False):
    if "nc" not in _CACHE:
        _CACHE["nc"] = _build()
    nc = _CACHE["nc"]
    in_maps = _shard_inputs(**inputs)
    res = bass_utils.run_bass_kernel_spmd(
        nc, in_maps, core_ids=list(range(NCORES)), trace=trace)
    out = np.stack([res.results[i]["out"] for i in range(NCORES)])
    return out.astype(np.float32), res


def kernel(**inputs):
    out, _ = _run(inputs, trace=False)
    return out
